# revision 1
# baseline (speedup 1.0000x reference)
"""GPT language model forward pass on 8 Trainium2 NeuronCores.

Sharding: sequence-parallel transformer with zigzag 128-token chunks
(core c of batch-group owns chunks {c%4, 7-c%4} of its batch -> balanced
causal attention), per-layer K/V AllGather within each batch's 4-core
group, final AllGather of the normalized activations, vocab-sharded tied
LM head (4000 vocab columns per core).

All matmul operands are fp16 (fp32 PSUM accumulation); residual stream,
layernorm statistics and softmax accumulation stay fp32.
"""
import numpy as np

import concourse.bass as bass
import concourse.mybir as mybir
import concourse.tile as tile
from concourse import bacc
from concourse import bass_utils
from concourse.masks import make_identity, make_upper_triangular

F32 = mybir.dt.float32
F16 = mybir.dt.float16
AF = mybir.ActivationFunctionType

C = 1024
H = 16
D = 64
L = 8
FF = 4096
VOC = 32000
B = 2
T = 1024
NCORE = 8
VS = VOC // NCORE          # 4000
TPC = 256                  # tokens per core (2 chunks of 128)
KV_K_ELEMS = C * TPC       # 262144 (K^T contribution, fp16)
VAUG_W = H * (D + 1)       # 1040 (V plus per-head ones column)
KV_V_ELEMS = TPC * VAUG_W  # 266240
KV_ELEMS = KV_K_ELEMS + KV_V_ELEMS
X_ELEMS = C * TPC
NEG = -1.0e9
ATT_SCALE = 0.125


def _ln_stats(nc, pool, xt):
    """Return (negmu, rstd) [128,1] f32 tiles for token-major xt [128, C]."""
    stats = pool.tile([128, 2, 6], F32, tag="lnstats")
    xv = xt.rearrange("p (a c) -> p a c", a=2)
    for sg in range(2):
        nc.vector.bn_stats(out=stats[:, sg, :], in_=xv[:, sg, :])
    mv = pool.tile([128, 2], F32, tag="lnmv")
    nc.vector.bn_aggr(out=mv, in_=stats)
    negmu = pool.tile([128, 1], F32, tag="lnnegmu")
    nc.vector.tensor_scalar_mul(out=negmu, in0=mv[:, 0:1], scalar1=-1.0)
    return mv, negmu


DBG_LAYERS = L
DBG_LMHEAD = True
DBG_KVGROUP = 4
DBG_ATTN = True
DBG_WO = True
DBG_FFN = True
DBG_AG = True


def _build_program():
    nc = bacc.Bacc("TRN2", target_bir_lowering=False, debug=False,
                   enable_asserts=True, num_devices=NCORE)

    # ---- inputs (per-core maps share most arrays) ----
    x0 = nc.dram_tensor("x0", [TPC, C], F32, kind="ExternalInput")
    kmask = nc.dram_tensor("kmask", [16], F32, kind="ExternalInput")
    wq = nc.dram_tensor("wq", [L, C, C], F16, kind="ExternalInput")
    wk = nc.dram_tensor("wk", [L, C, C], F16, kind="ExternalInput")
    wv = nc.dram_tensor("wv", [L, C, C], F16, kind="ExternalInput")
    wo = nc.dram_tensor("wo", [L, C, C], F16, kind="ExternalInput")
    w1n = nc.dram_tensor("w1n", [L, 8, 128, FF], F16, kind="ExternalInput")
    w2 = nc.dram_tensor("w2", [L, FF, C], F16, kind="ExternalInput")
    bo16 = nc.dram_tensor("bo16", [L, C], F16, kind="ExternalInput")
    b2_16 = nc.dram_tensor("b2_16", [L, C], F16, kind="ExternalInput")
    b1f = nc.dram_tensor("b1f", [L, FF], F32, kind="ExternalInput")
    ln1s = nc.dram_tensor("ln1s", [L, C], F32, kind="ExternalInput")
    ln1b = nc.dram_tensor("ln1b", [L, C], F32, kind="ExternalInput")
    ln2s = nc.dram_tensor("ln2s", [L, C], F32, kind="ExternalInput")
    ln2b = nc.dram_tensor("ln2b", [L, C], F32, kind="ExternalInput")
    lnfs = nc.dram_tensor("lnfs", [C], F32, kind="ExternalInput")
    lnfb = nc.dram_tensor("lnfb", [C], F32, kind="ExternalInput")
    embT = nc.dram_tensor("embT", [C, VS], F16, kind="ExternalInput")
    logits = nc.dram_tensor("logits", [B * T, VS], F32, kind="ExternalOutput")

    with tile.TileContext(nc) as tc:
        _body(nc, tc, locals())
    nc.compile()
    return nc


def _body(nc, tc, t):
    from contextlib import ExitStack
    ctx = ExitStack()
    with ctx:
        per = ctx.enter_context(tc.tile_pool(name="per", bufs=1))
        xpool = ctx.enter_context(tc.tile_pool(name="xpool", bufs=2))
        hpool = ctx.enter_context(tc.tile_pool(name="hpool", bufs=2))
        tmp = ctx.enter_context(tc.tile_pool(name="tmp", bufs=2))
        small = ctx.enter_context(tc.tile_pool(name="small", bufs=3))
        wsl = ctx.enter_context(tc.tile_pool(name="wsl", bufs=2))
        w1p = ctx.enter_context(tc.tile_pool(name="w1p", bufs=2))
        apool = ctx.enter_context(tc.tile_pool(name="apool", bufs=3))
        epool = ctx.enter_context(tc.tile_pool(name="epool", bufs=4))
        evp = ctx.enter_context(tc.tile_pool(name="evp", bufs=2))
        dram = ctx.enter_context(tc.tile_pool(name="dram", bufs=2,
                                              space="DRAM"))
        pbig = ctx.enter_context(tc.tile_pool(name="pbig", bufs=2,
                                              space="PSUM"))
        pmed = ctx.enter_context(tc.tile_pool(name="pmed", bufs=2,
                                              space="PSUM"))
        pacc = ctx.enter_context(tc.tile_pool(name="pacc", bufs=1,
                                              space="PSUM"))

        # ---- persistent constants ----
        ident = per.tile([128, 128], F32)
        make_identity(nc, ident)
        tril = per.tile([128, 128], F16)  # [k,q] keep k<=q
        make_upper_triangular(nc, tril, val=1.0, diag=True)
        tril4 = per.tile([128, 4, 128], F16)  # tril replicated x4 heads
        for hi in range(4):
            nc.vector.tensor_copy(out=tril4[:, hi, :], in_=tril)
        kmask_sb = per.tile([128, 16], F32)
        nc.gpsimd.dma_start(
            out=kmask_sb,
            in_=bass.AP(tensor=t["kmask"], offset=0, ap=[[0, 128], [1, 16]]))
        eps_t = per.tile([128, 1], F32)
        nc.vector.memset(eps_t, 1e-5)
        ones1 = per.tile([1, 128], F16)
        nc.vector.memset(ones1, 1.0)

        QT = per.tile([128, 8 * TPC], F16)       # Q^T fm, col=ct*256+tok
        KT_own = per.tile([128, 8 * TPC], F16)
        V_own = per.tile([128, 2, H, D + 1], F16)
        nc.vector.memset(V_own[:, :, :, D:D + 1], 1.0)
        KT_g = per.tile([128, 8, 8, 128], F16)   # [p, ct, j, tok]
        V_g = per.tile([128, 8, H, D + 1], F16)  # per j-chunk aug V
        # V_g needs per-j [128, 1040] -> [128, 8, 16, 65]
        nc.vector.memset(V_g[:, :, :, D:D + 1], 1.0)
        o_sb = per.tile([128, 2, C], F32)        # attention out, token-major
        oT = per.tile([128, 8 * TPC], F16)       # o^T fm
        xfT = per.tile([128, 8 * TPC], F16)      # final-LN x^T fm

        x_cur = []
        for ci in range(2):
            xt = xpool.tile([128, C], F32, tag=f"x{ci}")
            nc.sync.dma_start(out=xt, in_=t["x0"][ci * 128:(ci + 1) * 128, :])
            x_cur.append(xt)

        def ln_transpose(xt_pair, s_dram, b_dram, l, dstT, cast16=True):
            """LayerNorm (token-major) + transpose + scale/bias in fm."""
            scol = small.tile([128, 8], F32, tag="scol")
            bcol = small.tile([128, 8], F32, tag="bcol")
            src_s = s_dram[l] if l is not None else s_dram
            src_b = b_dram[l] if l is not None else b_dram
            nc.sync.dma_start(out=scol,
                              in_=src_s.rearrange("(a b) -> b a", b=128))
            nc.sync.dma_start(out=bcol,
                              in_=src_b.rearrange("(a b) -> b a", b=128))
            for ci in range(2):
                xt = xt_pair[ci]
                mv, negmu = _ln_stats(nc, small, xt)
                sq = small.tile([128, 1], F32, tag="lnsq")
                nc.scalar.activation(out=sq, in_=mv[:, 1:2], func=AF.Sqrt,
                                     bias=eps_t, scale=1.0)
                rstd = small.tile([128, 1], F32, tag="lnrstd")
                nc.vector.reciprocal(out=rstd, in_=sq)
                xn = tmp.tile([128, C], F32, tag="xn")
                nc.vector.tensor_scalar(out=xn, in0=xt, scalar1=negmu,
                                        scalar2=rstd,
                                        op0=mybir.AluOpType.add,
                                        op1=mybir.AluOpType.mult)
                for ct in range(8):
                    tp = pmed.tile([128, 128], F32, tag="med")
                    nc.tensor.transpose(tp, xn[:, ct * 128:(ct + 1) * 128],
                                        ident)
                    nc.vector.tensor_scalar(
                        out=dstT[:, ct * 256 + ci * 128:
                                 ct * 256 + ci * 128 + 128],
                        in0=tp, scalar1=scol[:, ct:ct + 1],
                        scalar2=bcol[:, ct:ct + 1],
                        op0=mybir.AluOpType.mult, op1=mybir.AluOpType.add)

        def load_w(w_dram, l):
            """One-DMA load of a [C, C] fp16 matrix into [128, 8, C]."""
            sl = wsl.tile([128, 8, C], F16, tag="wsl")
            nc.sync.dma_start(
                out=sl, in_=w_dram[l].rearrange("(a p) c -> p a c", p=128))
            return sl

        def proj_fm(w_dram, l, dstT):
            """dstT[feat, tok] (fp16) = W^T @ h^T, W [C,C] fm-major."""
            sl = load_w(w_dram, l)
            for grp in range(8):
                ps = pmed.tile([128, 256], F32, tag="med")
                for ct in range(8):
                    nc.tensor.matmul(
                        ps, sl[:, ct, grp * 128:(grp + 1) * 128],
                        hT[:, ct * 256:(ct + 1) * 256],
                        start=(ct == 0), stop=(ct == 7))
                nc.vector.tensor_copy(
                    out=dstT[:, grp * 256:(grp + 1) * 256], in_=ps)
            return sl

        for l in range(DBG_LAYERS):
            # ---- LN1 + h^T ----
            hT = hpool.tile([128, 8 * TPC], F16, tag="hT")
            ln_transpose(x_cur, t["ln1s"], t["ln1b"], l, hT)

            # ---- K, V projections first (feed the AllGather) ----
            proj_fm(t["wk"], l, KT_own)
            vsl = load_w(t["wv"], l)
            for ci in range(2):
                ps = pbig.tile([128, C], F32, tag="big")
                for half in range(2):
                    for ct in range(8):
                        nc.tensor.matmul(
                            ps[:, half * 512:(half + 1) * 512],
                            hT[:, ct * 256 + ci * 128:ct * 256 + ci * 128 + 128],
                            vsl[:, ct, half * 512:(half + 1) * 512],
                            start=(ct == 0), stop=(ct == 7))
                for half in range(2):
                    nc.vector.tensor_copy(
                        out=V_own[:, ci, half * 8:(half + 1) * 8, 0:D],
                        in_=ps[:, half * 512:(half + 1) * 512]
                        .rearrange("p (h d) -> p h d", h=8))

            # ---- launch K/V AllGather (per-batch groups of 4) ----
            ag_in = dram.tile([KV_ELEMS], F16, tag="agin")
            ag_out = dram.tile([DBG_KVGROUP * KV_ELEMS], F16, tag="agout")
            nc.sync.dma_start(
                out=ag_in[0:KV_K_ELEMS].rearrange("(a p c) -> p a c",
                                                  a=8, p=128),
                in_=KT_own[:].rearrange("p (a c) -> p a c", a=8))
            nc.sync.dma_start(
                out=ag_in[KV_K_ELEMS:KV_ELEMS].rearrange("(a p c) -> p a c",
                                                         a=2, p=128),
                in_=V_own[:].rearrange("p a h d -> p a (h d)"))
            kv_groups = ([[0, 1, 2, 3], [4, 5, 6, 7]] if DBG_KVGROUP == 4
                         else [list(range(8))])
            if DBG_AG:
                nc.gpsimd.collective_compute(
                    "AllGather", mybir.AluOpType.bypass,
                    replica_groups=kv_groups,
                    ins=[ag_in[:].opt()], outs=[ag_out[:].opt()])
            else:
                nc.sync.dma_start(out=ag_out[0:KV_ELEMS], in_=ag_in[:])

            # ---- Q projection (overlaps with AllGather) ----
            proj_fm(t["wq"], l, QT)

            # ---- load gathered K/V into global-chunk order ----
            for r in range(4):
                base = r * KV_ELEMS
                kv = ag_out[base:base + KV_K_ELEMS].rearrange(
                    "(a p c) -> a p c", a=8, p=128)
                vv = ag_out[base + KV_K_ELEMS:base + KV_ELEMS].rearrange(
                    "(a p c) -> a p c", a=2, p=128)
                for sub in range(2):
                    j = r if sub == 0 else 7 - r
                    nc.sync.dma_start(
                        out=KT_g[:, :, j, :],
                        in_=kv[:, :, sub * 128:(sub + 1) * 128]
                        .rearrange("a p c -> p a c"))
                    nc.sync.dma_start(
                        out=V_g[:, j, :, :],
                        in_=vv[sub, :, :].rearrange("p (h d) -> p h d", h=H))

            # ---- attention ----
            for h in range(H if DBG_ATTN else 0):
                po = (h % 2) * 64
                cth = h // 2
                o_aug = [pacc.tile([128, D + 1], F32, tag=f"oa{qc}",
                                   name=f"oaug{qc}_{l}_{h}")
                         for qc in range(2)]
                # local units: (qc, kc, masked)
                for (qc, kc, msk, first) in ((0, 0, True, True),
                                             (1, 1, True, True),
                                             (1, 0, False, False)):
                    ps = pmed.tile([128, 128], F32, tag="med")
                    nc.tensor.matmul(
                        ps,
                        KT_own[po:po + 64,
                               cth * 256 + kc * 128:cth * 256 + kc * 128 + 128],
                        QT[po:po + 64,
                           cth * 256 + qc * 128:cth * 256 + qc * 128 + 128],
                        start=True, stop=True)
                    e = epool.tile([128, 128], F16, tag="e")
                    nc.scalar.activation(out=e, in_=ps, func=AF.Exp,
                                         scale=ATT_SCALE)
                    if msk:
                        nc.vector.tensor_mul(out=e, in0=e, in1=tril)
                    nc.tensor.matmul(o_aug[qc], e,
                                     V_own[:, kc, h, :],
                                     start=first, stop=False)
                # remote units vs all 8 global chunks (masks from data)
                for j in range(8):
                    ps = pmed.tile([128, 256], F32, tag="med")
                    nc.tensor.matmul(
                        ps,
                        KT_g[po:po + 64, cth, j, :],
                        QT[po:po + 64, cth * 256:(cth + 1) * 256],
                        start=True, stop=True)
                    for qc in range(2):
                        e = epool.tile([128, 128], F16, tag="e")
                        nc.scalar.activation(
                            out=e, in_=ps[:, qc * 128:(qc + 1) * 128],
                            func=AF.Exp, scale=ATT_SCALE,
                            bias=kmask_sb[:, 8 * qc + j:8 * qc + j + 1])
                        nc.tensor.matmul(o_aug[qc], e, V_g[:, j, h, :],
                                         start=False, stop=(j == 7))
                for qc in range(2):
                    rec = small.tile([128, 1], F32, tag="rec")
                    nc.vector.reciprocal(out=rec, in_=o_aug[qc][:, D:D + 1])
                    nc.vector.tensor_scalar_mul(
                        out=o_sb[:, qc, h * D:(h + 1) * D],
                        in0=o_aug[qc][:, 0:D], scalar1=rec)

            # ---- output projection + residual ----
            for ci in range(2 if DBG_WO else 0):
                for ct in range(8):
                    tp = pmed.tile([128, 128], F32, tag="med")
                    nc.tensor.transpose(
                        tp, o_sb[:, ci, ct * 128:(ct + 1) * 128], ident)
                    nc.vector.tensor_copy(
                        out=oT[:, ct * 256 + ci * 128:ct * 256 + ci * 128 + 128],
                        in_=tp)
            wosl = load_w(t["wo"], l)
            bo_sb = small.tile([1, C], F16, tag="borow")
            nc.sync.dma_start(out=bo_sb, in_=t["bo16"][l:l + 1, :])
            x_new = []
            for ci in range(2):
                ps = pbig.tile([128, C], F32, tag="big")
                for half in range(2):
                    nc.tensor.matmul(ps[:, half * 512:(half + 1) * 512],
                                     ones1,
                                     bo_sb[:, half * 512:(half + 1) * 512],
                                     start=True, stop=False)
                    for ct in range(8):
                        nc.tensor.matmul(
                            ps[:, half * 512:(half + 1) * 512],
                            oT[:, ct * 256 + ci * 128:ct * 256 + ci * 128 + 128],
                            wosl[:, ct, half * 512:(half + 1) * 512],
                            start=False, stop=(ct == 7))
                xt = xpool.tile([128, C], F32, tag=f"x{ci}")
                nc.vector.tensor_tensor(out=xt, in0=ps, in1=x_cur[ci],
                                        op=mybir.AluOpType.add)
                x_new.append(xt)
            if x_new:
                x_cur = x_new

            # ---- LN2 + FFN ----
            if not DBG_FFN:
                continue
            hT = hpool.tile([128, 8 * TPC], F16, tag="hT")
            ln_transpose(x_cur, t["ln2s"], t["ln2b"], l, hT)
            b1col = small.tile([128, FF // 128], F32, tag="b1col")
            nc.sync.dma_start(out=b1col,
                              in_=t["b1f"][l].rearrange("(a b) -> b a", b=128))
            b2_sb = small.tile([1, C], F16, tag="b2row")
            nc.sync.dma_start(out=b2_sb, in_=t["b2_16"][l:l + 1, :])
            ps_f2 = []
            for ci in range(2):
                ps = pbig.tile([128, C], F32, tag="big")
                for half in range(2):
                    nc.tensor.matmul(ps[:, half * 512:(half + 1) * 512],
                                     ones1,
                                     b2_sb[:, half * 512:(half + 1) * 512],
                                     start=True, stop=False)
                ps_f2.append(ps)
            for fc in range(8):
                w1t = w1p.tile([128, 8, 512], F16, tag="w1t")
                nc.sync.dma_start(
                    out=w1t,
                    in_=t["w1n"][l, :, :, fc * 512:(fc + 1) * 512]
                    .rearrange("a p c -> p a c"))
                w2t = w1p.tile([128, 4, C], F16, tag="w2t")
                nc.sync.dma_start(
                    out=w2t,
                    in_=t["w2"][l, fc * 512:(fc + 1) * 512, :]
                    .rearrange("(a p) c -> p a c", p=128))
                for fs in range(4):
                    f = fc * 4 + fs
                    ps1 = pmed.tile([128, 256], F32, tag="med")
                    for ct in range(8):
                        nc.tensor.matmul(
                            ps1, w1t[:, ct, fs * 128:(fs + 1) * 128],
                            hT[:, ct * 256:(ct + 1) * 256],
                            start=(ct == 0), stop=(ct == 7))
                    aT = apool.tile([128, 256], F16, tag="aT")
                    nc.scalar.activation(out=aT, in_=ps1, func=AF.Relu,
                                         bias=b1col[:, f:f + 1], scale=1.0)
                    for ci in range(2):
                        for half in range(2):
                            nc.tensor.matmul(
                                ps_f2[ci][:, half * 512:(half + 1) * 512],
                                aT[:, ci * 128:(ci + 1) * 128],
                                w2t[:, fs, half * 512:(half + 1) * 512],
                                start=False, stop=(f == FF // 128 - 1))
            x_new = []
            for ci in range(2):
                xt = xpool.tile([128, C], F32, tag=f"x{ci}")
                nc.vector.tensor_tensor(out=xt, in0=ps_f2[ci], in1=x_cur[ci],
                                        op=mybir.AluOpType.add)
                x_new.append(xt)
            x_cur = x_new

        # ---- final layernorm -> x^T fm fp16 -> AllGather all 8 cores ----
        ln_transpose(x_cur, t["lnfs"], t["lnfb"], None, xfT)
        agx_in = dram.tile([X_ELEMS], F16, tag="agxin")
        agx_out = dram.tile([NCORE * X_ELEMS], F16, tag="agxout",
                            addr_space="Shared")
        nc.sync.dma_start(
            out=agx_in[:].rearrange("(a p c) -> p a c", a=8, p=128),
            in_=xfT[:].rearrange("p (a c) -> p a c", a=8))
        nc.gpsimd.collective_compute(
            "AllGather", mybir.AluOpType.bypass,
            replica_groups=[list(range(NCORE))],
            ins=[agx_in[:].opt()], outs=[agx_out[:].opt()])

        # ---- LM head: logits[2048, 4000] = X^T.T @ embT ----
        # stream gathered X^T per (vg, tt) instead of holding all of it
        for vg in range(8 if DBG_LMHEAD else 0):
            evt = evp.tile([128, 8, 500], F16, tag="evt")
            nc.sync.dma_start(
                out=evt,
                in_=t["embT"].rearrange("(a p) v -> p a v", p=128)
                [:, :, vg * 500:(vg + 1) * 500])
            for tt in range(16):
                r = tt // 2
                xtt = evp.tile([128, 8, 128], F16, tag="xtt")
                nc.sync.dma_start(
                    out=xtt,
                    in_=agx_out[r * X_ELEMS:(r + 1) * X_ELEMS]
                    .rearrange("(a p c) -> p a c", a=8, p=128)
                    [:, :, (tt % 2) * 128:(tt % 2) * 128 + 128])
                ps = pmed.tile([128, 500], F32, tag="med")
                for ct in range(8):
                    nc.tensor.matmul(ps, xtt[:, ct, :], evt[:, ct, :],
                                     start=(ct == 0), stop=(ct == 7))
                lstage = apool.tile([128, 500], F32, tag="lstage")
                nc.vector.tensor_copy(out=lstage, in_=ps)
                nc.sync.dma_start(
                    out=t["logits"][tt * 128:(tt + 1) * 128,
                                    vg * 500:(vg + 1) * 500],
                    in_=lstage)


_PROG = None


def _get_program():
    global _PROG
    if _PROG is None:
        _PROG = _build_program()
    return _PROG


def _host_inputs(idx, tok_emb, pos_emb, ln1_s, ln1_b, Wq, Wk, Wv, Wo, bo,
                 ln2_s, ln2_b, W1, b1, W2, b2, lnf_s, lnf_b):
    f16 = np.float16
    emb = (tok_emb[idx] + pos_emb[None, :, :]).astype(np.float32)  # (B,T,C)
    wq_t = np.ascontiguousarray(
        np.transpose(Wq, (0, 2, 1, 3)).reshape(L, C, C).astype(f16))
    wk_t = np.ascontiguousarray(
        np.transpose(Wk, (0, 2, 1, 3)).reshape(L, C, C).astype(f16))
    wv_t = np.ascontiguousarray(
        np.transpose(Wv, (0, 2, 1, 3)).reshape(L, C, C).astype(f16))
    w1nk = np.ascontiguousarray(W1.reshape(L, 8, 128, FF).astype(f16))
    shared = {
        "wq": wq_t, "wk": wk_t, "wv": wv_t,
        "wo": np.ascontiguousarray(Wo.astype(f16)),
        "w1n": w1nk,
        "w2": np.ascontiguousarray(W2.astype(f16)),
        "bo16": bo.astype(f16), "b2_16": b2.astype(f16),
        "b1f": b1.astype(np.float32),
        "ln1s": ln1_s.astype(np.float32), "ln1b": ln1_b.astype(np.float32),
        "ln2s": ln2_s.astype(np.float32), "ln2b": ln2_b.astype(np.float32),
        "lnfs": lnf_s.astype(np.float32), "lnfb": lnf_b.astype(np.float32),
    }
    embT_full = np.ascontiguousarray(tok_emb.T.astype(f16))  # (C, VOC)
    in_maps = []
    for c in range(NCORE):
        b = c // 4
        c4 = c % 4
        g1, g2 = c4, 7 - c4
        x0 = np.concatenate([emb[b, g1 * 128:(g1 + 1) * 128],
                             emb[b, g2 * 128:(g2 + 1) * 128]], axis=0)
        km = np.full(16, NEG, np.float32)
        for j in range(8):
            if j < g1:
                km[j] = 0.0
            if j < g2 and j != g1:
                km[8 + j] = 0.0
        m = dict(shared)
        m["x0"] = np.ascontiguousarray(x0.astype(np.float32))
        m["kmask"] = km
        m["embT"] = np.ascontiguousarray(embT_full[:, c * VS:(c + 1) * VS])
        in_maps.append(m)
    return in_maps


def _assemble(results):
    """results: list of per-core dicts with 'logits' [2048, VS] in gathered
    token order (rank-major zigzag chunks). Returns (B, T, VOC) f32."""
    out = np.empty((B, T, VOC), np.float32)
    for c in range(NCORE):
        lg = results[c]["logits"]  # rows: rank r block 256 = chunks (g1,g2)
        vsl = slice(c * VS, (c + 1) * VS)
        for r in range(NCORE):
            rb = r // 4
            r4 = r % 4
            g1, g2 = r4, 7 - r4
            blk = lg[r * 256:(r + 1) * 256]
            out[rb, g1 * 128:(g1 + 1) * 128, vsl] = blk[0:128]
            out[rb, g2 * 128:(g2 + 1) * 128, vsl] = blk[128:256]
    return out


def kernel(**inputs):
    nc = _get_program()
    in_maps = _host_inputs(**inputs)
    res = bass_utils.run_bass_kernel_spmd(
        nc, in_maps, core_ids=list(range(NCORE)))
    return _assemble(res.results)


if __name__ == "__main__":
    import reference as R
    inp = {k: np.asarray(v) for k, v in R.setup_inputs().items()}
    out = kernel(**inp)
    exp = np.asarray(R.reference(**inp))
    err = np.abs(out - exp)
    print("absmax expected:", np.abs(exp).max())
    print("max abs err:", err.max(),
          "rel:", err.max() / np.abs(exp).max())



# revision 2
# speedup vs baseline: 4927.2543x; 4927.2543x over previous
"""GPT language model forward pass on 8 Trainium2 NeuronCores.

Sharding: sequence-parallel transformer with zigzag 128-token chunks
(core c of batch-group owns chunks {c%4, 7-c%4} of its batch -> balanced
causal attention), per-layer K then V AllGather within each batch's
4-core group, token-sharded tied LM head (each core computes its own 256
tokens x full 32000 vocab, streaming the embedding matrix).

All matmul operands are fp16 (fp32 PSUM accumulation); residual stream,
layernorm statistics and softmax accumulation stay fp32. LayerNorm scales
are folded into the following weight matrices on the host; LN biases are
applied as bias columns fused into the PSUM->SBUF copies (or bias-row
matmuls for token-major outputs).

Attention computes scores for 8-head groups into a [128, 8, 128] PSUM
tile per (j-chunk, query-chunk): one wide exp per tile with the causal
chunk mask applied via the activation bias; K=64 QK^T matmuls are issued
in row-group pairs (partitions 0-63 / 64-127) so they run concurrently
on the PE sub-arrays.
"""
import numpy as np

import concourse.bass as bass
import concourse.mybir as mybir
import concourse.tile as tile
from concourse import bacc
from concourse import bass_utils
from concourse.masks import make_identity, make_upper_triangular

F32 = mybir.dt.float32
F16 = mybir.dt.float16
AF = mybir.ActivationFunctionType

C = 1024
H = 16
D = 64
L = 8
FF = 4096
VOC = 32000
B = 2
T = 1024
NCORE = 8
TPC = 256                  # tokens per core (2 chunks of 128)
K_ELEMS = C * TPC          # 262144 (K^T contribution, fp16)
VAUG_W = H * (D + 1)       # 1040 (V plus per-head ones column)
V_ELEMS = TPC * VAUG_W     # 266240
NEG = -1.0e9
ATT_SCALE = 0.125
VCH = 500                  # vocab chunk per LM-head psum tile
NVCH = VOC // VCH          # 64


def _build_program():
    nc = bacc.Bacc("TRN2", target_bir_lowering=False, debug=False,
                   enable_asserts=True, num_devices=NCORE)

    # ---- inputs (per-core maps share most arrays) ----
    x0 = nc.dram_tensor("x0", [TPC, C], F32, kind="ExternalInput")
    kmask = nc.dram_tensor("kmask", [16], F32, kind="ExternalInput")
    wq = nc.dram_tensor("wq", [L, C, C], F16, kind="ExternalInput")
    wk = nc.dram_tensor("wk", [L, C, C], F16, kind="ExternalInput")
    wv = nc.dram_tensor("wv", [L, C, C], F16, kind="ExternalInput")
    wo = nc.dram_tensor("wo", [L, C, C], F16, kind="ExternalInput")
    w1n = nc.dram_tensor("w1n", [L, 8, 128, FF], F16, kind="ExternalInput")
    w2 = nc.dram_tensor("w2", [L, FF, C], F16, kind="ExternalInput")
    bo16 = nc.dram_tensor("bo16", [L, C], F16, kind="ExternalInput")
    b2_16 = nc.dram_tensor("b2_16", [L, C], F16, kind="ExternalInput")
    vb16 = nc.dram_tensor("vb16", [L, C], F16, kind="ExternalInput")
    qbias = nc.dram_tensor("qbias", [L, C], F32, kind="ExternalInput")
    kbias = nc.dram_tensor("kbias", [L, C], F32, kind="ExternalInput")
    b1f = nc.dram_tensor("b1f", [L, FF], F32, kind="ExternalInput")
    embT = nc.dram_tensor("embT", [C, VOC], F16, kind="ExternalInput")
    logits = nc.dram_tensor("logits", [TPC, VOC], F16, kind="ExternalOutput")

    with tile.TileContext(nc) as tc:
        _body(nc, tc, locals())
    nc.compile()
    return nc


def _body(nc, tc, t):
    from contextlib import ExitStack
    ctx = ExitStack()
    with ctx:
        per = ctx.enter_context(tc.tile_pool(name="per", bufs=1))
        xpool = ctx.enter_context(tc.tile_pool(name="xpool", bufs=2))
        hpool = ctx.enter_context(tc.tile_pool(name="hpool", bufs=2))
        tmp = ctx.enter_context(tc.tile_pool(name="tmp", bufs=2))
        small = ctx.enter_context(tc.tile_pool(name="small", bufs=3))
        wsl = ctx.enter_context(tc.tile_pool(name="wsl", bufs=2))
        w1p = ctx.enter_context(tc.tile_pool(name="w1p", bufs=2))
        apool = ctx.enter_context(tc.tile_pool(name="apool", bufs=3))
        epool = ctx.enter_context(tc.tile_pool(name="epool", bufs=3))
        evp = ctx.enter_context(tc.tile_pool(name="evp", bufs=2))
        dram = ctx.enter_context(tc.tile_pool(name="dram", bufs=2,
                                              space="DRAM"))
        pbig = ctx.enter_context(tc.tile_pool(name="pbig", bufs=2,
                                              space="PSUM"))
        pmed = ctx.enter_context(tc.tile_pool(name="pmed", bufs=4,
                                              space="PSUM"))

        # ---- persistent constants ----
        ident = per.tile([128, 128], F32)
        make_identity(nc, ident)
        tril = per.tile([128, 128], F16)  # [k,q] keep k<=q
        make_upper_triangular(nc, tril, val=1.0, diag=True)
        tril8 = per.tile([128, 8, 128], F16)  # tril replicated x8 heads
        for hi in range(8):
            nc.vector.tensor_copy(out=tril8[:, hi, :], in_=tril)
        kmask_sb = per.tile([128, 16], F32)
        nc.gpsimd.dma_start(
            out=kmask_sb,
            in_=bass.AP(tensor=t["kmask"], offset=0, ap=[[0, 128], [1, 16]]))
        eps_t = per.tile([128, 1], F32)
        nc.vector.memset(eps_t, 1e-5)
        ones1 = per.tile([1, 128], F16)
        nc.vector.memset(ones1, 1.0)

        QT = per.tile([128, 8 * TPC], F16)       # Q^T fm, col=ct*256+tok
        KT_own = per.tile([128, 8 * TPC], F16)
        V_own = per.tile([128, 2, H, D + 1], F16)
        nc.vector.memset(V_own[:, :, :, D:D + 1], 1.0)
        KT_g = per.tile([128, 8, 8, 128], F16)   # [p, ct, j, tok]
        V_g = per.tile([128, 8, H, D + 1], F16)  # per j-chunk aug V
        nc.vector.memset(V_g[:, :, :, D:D + 1], 1.0)
        o_sb = per.tile([128, 2, C], F32)        # attention out, token-major
        oT = per.tile([128, 8 * TPC], F16)       # o^T fm
        xfT = per.tile([128, 8 * TPC], F16)      # final-LN x^T fm

        x_cur = []
        for ci in range(2):
            xt = xpool.tile([128, C], F32, tag=f"x{ci}")
            nc.sync.dma_start(out=xt, in_=t["x0"][ci * 128:(ci + 1) * 128, :])
            x_cur.append(xt)

        def ln_transpose(xt_pair, dstT):
            """LayerNorm (token-major, scale/bias folded out) + transpose."""
            for ci in range(2):
                xt = xt_pair[ci]
                stats = small.tile([128, 2, 6], F32, tag="lnstats")
                xv = xt.rearrange("p (a c) -> p a c", a=2)
                for sg in range(2):
                    nc.vector.bn_stats(out=stats[:, sg, :], in_=xv[:, sg, :])
                mv = small.tile([128, 2], F32, tag="lnmv")
                nc.vector.bn_aggr(out=mv, in_=stats)
                negmu = small.tile([128, 1], F32, tag="lnnegmu")
                nc.vector.tensor_scalar_mul(out=negmu, in0=mv[:, 0:1],
                                            scalar1=-1.0)
                rstd = small.tile([128, 1], F32, tag="lnrstd")
                nc.scalar.activation(out=rstd, in_=mv[:, 1:2], func=AF.Rsqrt,
                                     bias=eps_t, scale=1.0)
                xn = tmp.tile([128, C], F32, tag="xn")
                nc.vector.tensor_scalar(out=xn, in0=xt, scalar1=negmu,
                                        scalar2=rstd,
                                        op0=mybir.AluOpType.add,
                                        op1=mybir.AluOpType.mult)
                for ct in range(8):
                    tp = pmed.tile([128, 128], F32, tag="med")
                    nc.tensor.transpose(tp, xn[:, ct * 128:(ct + 1) * 128],
                                        ident)
                    nc.vector.tensor_copy(
                        out=dstT[:, ct * 256 + ci * 128:
                                 ct * 256 + ci * 128 + 128],
                        in_=tp)

        def load_w(w_dram, l):
            """One-DMA load of a [C, C] fp16 matrix into [128, 8, C]."""
            sl = wsl.tile([128, 8, C], F16, tag="wsl")
            nc.sync.dma_start(
                out=sl, in_=w_dram[l].rearrange("(a p) c -> p a c", p=128))
            return sl

        def proj_fm(w_dram, b_dram, l, dstT, hT):
            """dstT[feat, tok] (fp16) = W^T @ h^T + bias col, fm-major."""
            sl = load_w(w_dram, l)
            bcol = small.tile([128, 8], F32, tag="pbcol")
            nc.sync.dma_start(out=bcol,
                              in_=b_dram[l].rearrange("(a b) -> b a", b=128))
            for grp in range(8):
                ps = pmed.tile([128, 256], F32, tag="med")
                for ct in range(8):
                    nc.tensor.matmul(
                        ps, sl[:, ct, grp * 128:(grp + 1) * 128],
                        hT[:, ct * 256:(ct + 1) * 256],
                        start=(ct == 0), stop=(ct == 7))
                nc.vector.tensor_scalar(
                    out=dstT[:, grp * 256:(grp + 1) * 256], in0=ps,
                    scalar1=bcol[:, grp:grp + 1],
                    op0=mybir.AluOpType.add)
            return sl

        for l in range(L):
            # ---- LN1 + h^T ----
            hT = hpool.tile([128, 8 * TPC], F16, tag="hT")
            ln_transpose(x_cur, hT)

            # ---- K, V projections first (feed the AllGathers) ----
            proj_fm(t["wk"], t["kbias"], l, KT_own, hT)
            vsl = load_w(t["wv"], l)
            vb_sb = small.tile([1, C], F16, tag="vbrow")
            nc.sync.dma_start(out=vb_sb, in_=t["vb16"][l:l + 1, :])
            for ci in range(2):
                ps = pbig.tile([128, C], F32, tag="big")
                for half in range(2):
                    nc.tensor.matmul(ps[:, half * 512:(half + 1) * 512],
                                     ones1,
                                     vb_sb[:, half * 512:(half + 1) * 512],
                                     start=True, stop=False)
                    for ct in range(8):
                        nc.tensor.matmul(
                            ps[:, half * 512:(half + 1) * 512],
                            hT[:, ct * 256 + ci * 128:ct * 256 + ci * 128 + 128],
                            vsl[:, ct, half * 512:(half + 1) * 512],
                            start=False, stop=(ct == 7))
                for half in range(2):
                    nc.vector.tensor_copy(
                        out=V_own[:, ci, half * 8:(half + 1) * 8, 0:D],
                        in_=ps[:, half * 512:(half + 1) * 512]
                        .rearrange("p (h d) -> p h d", h=8))

            # ---- launch K then V AllGather (per-batch groups of 4) ----
            kv_groups = [[0, 1, 2, 3], [4, 5, 6, 7]]
            agk_in = dram.tile([K_ELEMS], F16, tag="agkin")
            agk_out = dram.tile([4 * K_ELEMS], F16, tag="agkout")
            nc.sync.dma_start(
                out=agk_in[:].rearrange("(a p c) -> p a c", a=8, p=128),
                in_=KT_own[:].rearrange("p (a c) -> p a c", a=8))
            nc.gpsimd.collective_compute(
                "AllGather", mybir.AluOpType.bypass,
                replica_groups=kv_groups,
                ins=[agk_in[:].opt()], outs=[agk_out[:].opt()])
            agv_in = dram.tile([V_ELEMS], F16, tag="agvin")
            agv_out = dram.tile([4 * V_ELEMS], F16, tag="agvout")
            nc.sync.dma_start(
                out=agv_in[:].rearrange("(a p c) -> p a c", a=2, p=128),
                in_=V_own[:].rearrange("p a h d -> p a (h d)"))
            nc.gpsimd.collective_compute(
                "AllGather", mybir.AluOpType.bypass,
                replica_groups=kv_groups,
                ins=[agv_in[:].opt()], outs=[agv_out[:].opt()])

            # ---- Q projection (overlaps with AllGather) ----
            proj_fm(t["wq"], t["qbias"], l, QT, hT)

            # ---- load gathered K/V into global-chunk order ----
            for r in range(4):
                kv = agk_out[r * K_ELEMS:(r + 1) * K_ELEMS].rearrange(
                    "(a p c) -> a p c", a=8, p=128)
                vv = agv_out[r * V_ELEMS:(r + 1) * V_ELEMS].rearrange(
                    "(a p c) -> a p c", a=2, p=128)
                for sub in range(2):
                    j = r if sub == 0 else 7 - r
                    nc.sync.dma_start(
                        out=KT_g[:, :, j, :],
                        in_=kv[:, :, sub * 128:(sub + 1) * 128]
                        .rearrange("a p c -> p a c"))
                    nc.sync.dma_start(
                        out=V_g[:, j, :, :],
                        in_=vv[sub, :, :].rearrange("p (h d) -> p h d", h=H))

            # ---- attention: 2 passes x 8 heads, qc-separate j-loops ----
            for p in range(2):
                for qc in range(2):
                    o_aug = [pmed.tile([128, 4, D + 1], F32, tag="oaug",
                                       name=f"oaug{p}_{qc}_{q4}_{l}")
                             for q4 in range(2)]
                    # unit list: (kind, kc_or_j, masked)
                    units = ([("loc", 0, True)] if qc == 0 else
                             [("loc", 1, True), ("loc", 0, False)])
                    units += [("rem", j, False) for j in range(8)]
                    e_tiles = []
                    for ui, (kind, kj, msk) in enumerate(units):
                        ps = pbig.tile([128, 8, 128], F32, tag="big")
                        for i in range(8):
                            h = p * 8 + i
                            po = (h % 2) * 64
                            cth = h // 2
                            if kind == "loc":
                                lhs = KT_own[po:po + 64,
                                             cth * 256 + kj * 128:
                                             cth * 256 + kj * 128 + 128]
                            else:
                                lhs = KT_g[po:po + 64, cth, kj, :]
                            nc.tensor.matmul(
                                ps[:, i, :], lhs,
                                QT[po:po + 64,
                                   cth * 256 + qc * 128:
                                   cth * 256 + qc * 128 + 128],
                                start=True, stop=True)
                        e = epool.tile([128, 8, 128], F16, tag="e")
                        if kind == "rem":
                            nc.scalar.activation(
                                out=e, in_=ps, func=AF.Exp, scale=ATT_SCALE,
                                bias=kmask_sb[:, 8 * qc + kj:8 * qc + kj + 1])
                        else:
                            nc.scalar.activation(out=e, in_=ps, func=AF.Exp,
                                                 scale=ATT_SCALE)
                            if msk:
                                nc.vector.tensor_mul(out=e, in0=e, in1=tril8)
                        e_tiles.append(e)
                        # software-pipelined AV: trail QK/exp by one unit
                        if ui > 0:
                            _av(nc, p, qc, units[ui - 1], e_tiles[ui - 1],
                                o_aug, V_own, V_g, first=(ui == 1),
                                last=False)
                    _av(nc, p, qc, units[-1], e_tiles[-1], o_aug,
                        V_own, V_g, first=False, last=True)
                    # normalize -> o_sb token-major
                    for q4 in range(2):
                        rec = small.tile([128, 4], F32, tag="rec")
                        nc.vector.reciprocal(
                            out=rec, in_=o_aug[q4][:, :, D:D + 1]
                            .rearrange("p h one -> p (h one)"))
                        for hq in range(4):
                            h = p * 8 + q4 * 4 + hq
                            nc.vector.tensor_scalar_mul(
                                out=o_sb[:, qc, h * D:(h + 1) * D],
                                in0=o_aug[q4][:, hq, 0:D],
                                scalar1=rec[:, hq:hq + 1])

            # ---- o^T + output projection + residual ----
            for ci in range(2):
                for ct in range(8):
                    tp = pmed.tile([128, 128], F32, tag="med")
                    nc.tensor.transpose(
                        tp, o_sb[:, ci, ct * 128:(ct + 1) * 128], ident)
                    nc.vector.tensor_copy(
                        out=oT[:, ct * 256 + ci * 128:ct * 256 + ci * 128 + 128],
                        in_=tp)
            wosl = load_w(t["wo"], l)
            bo_sb = small.tile([1, C], F16, tag="borow")
            nc.sync.dma_start(out=bo_sb, in_=t["bo16"][l:l + 1, :])
            x_new = []
            for ci in range(2):
                ps = pbig.tile([128, C], F32, tag="big")
                for half in range(2):
                    nc.tensor.matmul(ps[:, half * 512:(half + 1) * 512],
                                     ones1,
                                     bo_sb[:, half * 512:(half + 1) * 512],
                                     start=True, stop=False)
                    for ct in range(8):
                        nc.tensor.matmul(
                            ps[:, half * 512:(half + 1) * 512],
                            oT[:, ct * 256 + ci * 128:ct * 256 + ci * 128 + 128],
                            wosl[:, ct, half * 512:(half + 1) * 512],
                            start=False, stop=(ct == 7))
                xt = xpool.tile([128, C], F32, tag=f"x{ci}")
                nc.vector.tensor_tensor(out=xt, in0=ps, in1=x_cur[ci],
                                        op=mybir.AluOpType.add)
                x_new.append(xt)
            x_cur = x_new

            # ---- LN2 + FFN ----
            hT = hpool.tile([128, 8 * TPC], F16, tag="hT")
            ln_transpose(x_cur, hT)
            b1col = small.tile([128, FF // 128], F32, tag="b1col")
            nc.sync.dma_start(out=b1col,
                              in_=t["b1f"][l].rearrange("(a b) -> b a", b=128))
            b2_sb = small.tile([1, C], F16, tag="b2row")
            nc.sync.dma_start(out=b2_sb, in_=t["b2_16"][l:l + 1, :])
            ps_f2 = []
            for ci in range(2):
                ps = pbig.tile([128, C], F32, tag="big")
                for half in range(2):
                    nc.tensor.matmul(ps[:, half * 512:(half + 1) * 512],
                                     ones1,
                                     b2_sb[:, half * 512:(half + 1) * 512],
                                     start=True, stop=False)
                ps_f2.append(ps)
            for fc in range(8):
                w1t = w1p.tile([128, 8, 512], F16, tag="w1t")
                nc.sync.dma_start(
                    out=w1t,
                    in_=t["w1n"][l, :, :, fc * 512:(fc + 1) * 512]
                    .rearrange("a p c -> p a c"))
                w2t = w1p.tile([128, 4, C], F16, tag="w2t")
                nc.sync.dma_start(
                    out=w2t,
                    in_=t["w2"][l, fc * 512:(fc + 1) * 512, :]
                    .rearrange("(a p) c -> p a c", p=128))
                for fs in range(4):
                    f = fc * 4 + fs
                    ps1 = pmed.tile([128, 256], F32, tag="med")
                    for ct in range(8):
                        nc.tensor.matmul(
                            ps1, w1t[:, ct, fs * 128:(fs + 1) * 128],
                            hT[:, ct * 256:(ct + 1) * 256],
                            start=(ct == 0), stop=(ct == 7))
                    aT = apool.tile([128, 256], F16, tag="aT")
                    nc.scalar.activation(out=aT, in_=ps1, func=AF.Relu,
                                         bias=b1col[:, f:f + 1], scale=1.0)
                    for ci in range(2):
                        for half in range(2):
                            nc.tensor.matmul(
                                ps_f2[ci][:, half * 512:(half + 1) * 512],
                                aT[:, ci * 128:(ci + 1) * 128],
                                w2t[:, fs, half * 512:(half + 1) * 512],
                                start=False, stop=(f == FF // 128 - 1))
            x_new = []
            for ci in range(2):
                xt = xpool.tile([128, C], F32, tag=f"x{ci}")
                nc.vector.tensor_tensor(out=xt, in0=ps_f2[ci], in1=x_cur[ci],
                                        op=mybir.AluOpType.add)
                x_new.append(xt)
            x_cur = x_new

        # ---- final layernorm -> x^T fm fp16 (lnf scale folded into embT) --
        ln_transpose(x_cur, xfT)

        # ---- LM head: logits[256, VOC] = X^T.T @ embT, token-sharded ----
        for vc in range(NVCH):
            evt = evp.tile([128, 8, VCH], F16, tag="evt")
            nc.sync.dma_start(
                out=evt,
                in_=t["embT"].rearrange("(a p) v -> p a v", p=128)
                [:, :, vc * VCH:(vc + 1) * VCH])
            for qc in range(2):
                ps = pmed.tile([128, VCH], F32, tag="lmps")
                for ct in range(8):
                    nc.tensor.matmul(
                        ps, xfT[:, ct * 256 + qc * 128:ct * 256 + qc * 128 + 128],
                        evt[:, ct, :],
                        start=(ct == 0), stop=(ct == 7))
                lstage = apool.tile([128, VCH], F16, tag="lstage")
                nc.vector.tensor_copy(out=lstage, in_=ps)
                nc.sync.dma_start(
                    out=t["logits"][qc * 128:(qc + 1) * 128,
                                    vc * VCH:(vc + 1) * VCH],
                    in_=lstage)


def _av(nc, p, qc, unit, e, o_aug, V_own, V_g, first, last):
    kind, kj, _ = unit
    for i in range(8):
        h = p * 8 + i
        v = V_own[:, kj, h, :] if kind == "loc" else V_g[:, kj, h, :]
        nc.tensor.matmul(o_aug[i // 4][:, i % 4, :], e[:, i, :], v,
                         start=first and i < 8, stop=last)


_PROG = None


def _get_program():
    global _PROG
    if _PROG is None:
        _PROG = _build_program()
    return _PROG


def _host_inputs(idx, tok_emb, pos_emb, ln1_s, ln1_b, Wq, Wk, Wv, Wo, bo,
                 ln2_s, ln2_b, W1, b1, W2, b2, lnf_s, lnf_b):
    f16 = np.float16
    emb = (tok_emb[idx] + pos_emb[None, :, :]).astype(np.float32)  # (B,T,C)
    wq_f = np.transpose(Wq, (0, 2, 1, 3)).reshape(L, C, C)
    wk_f = np.transpose(Wk, (0, 2, 1, 3)).reshape(L, C, C)
    wv_f = np.transpose(Wv, (0, 2, 1, 3)).reshape(L, C, C)
    s1 = ln1_s[:, :, None]
    s2 = ln2_s[:, :, None]
    shared = {
        "wq": np.ascontiguousarray((wq_f * s1).astype(f16)),
        "wk": np.ascontiguousarray((wk_f * s1).astype(f16)),
        "wv": np.ascontiguousarray((wv_f * s1).astype(f16)),
        "wo": np.ascontiguousarray(Wo.astype(f16)),
        "w1n": np.ascontiguousarray((W1 * s2).reshape(L, 8, 128, FF)
                                    .astype(f16)),
        "w2": np.ascontiguousarray(W2.astype(f16)),
        "bo16": bo.astype(f16), "b2_16": b2.astype(f16),
        "vb16": np.einsum("lc,lcd->ld", ln1_b, wv_f).astype(f16),
        "qbias": np.einsum("lc,lcd->ld", ln1_b, wq_f).astype(np.float32),
        "kbias": np.einsum("lc,lcd->ld", ln1_b, wk_f).astype(np.float32),
        "b1f": (b1 + np.einsum("lc,lcf->lf", ln2_b, W1)).astype(np.float32),
        "embT": np.ascontiguousarray(
            (tok_emb * lnf_s[None, :]).T.astype(f16)),
    }
    in_maps = []
    for c in range(NCORE):
        b = c // 4
        c4 = c % 4
        g1, g2 = c4, 7 - c4
        x0 = np.concatenate([emb[b, g1 * 128:(g1 + 1) * 128],
                             emb[b, g2 * 128:(g2 + 1) * 128]], axis=0)
        km = np.full(16, NEG, np.float32)
        for j in range(8):
            if j < g1:
                km[j] = 0.0
            if j < g2 and j != g1:
                km[8 + j] = 0.0
        m = dict(shared)
        m["x0"] = np.ascontiguousarray(x0.astype(np.float32))
        m["kmask"] = km
        in_maps.append(m)
    return in_maps, lnf_b


def _assemble(results, lnf_b, tok_emb):
    """results: per-core dicts with fp16 'logits' [256, VOC] for the core's
    two zigzag chunks. Returns (B, T, VOC) f32."""
    out = np.empty((B, T, VOC), np.float32)
    for c in range(NCORE):
        lg = results[c]["logits"].astype(np.float32)
        b = c // 4
        g1, g2 = c % 4, 7 - c % 4
        out[b, g1 * 128:(g1 + 1) * 128] = lg[0:128]
        out[b, g2 * 128:(g2 + 1) * 128] = lg[128:256]
    if np.any(lnf_b):
        out += (lnf_b @ tok_emb.T)[None, None, :]
    return out


def kernel(**inputs):
    nc = _get_program()
    in_maps, lnf_b = _host_inputs(**inputs)
    res = bass_utils.run_bass_kernel_spmd(
        nc, in_maps, core_ids=list(range(NCORE)))
    return _assemble(res.results, lnf_b, inputs["tok_emb"])


if __name__ == "__main__":
    import reference as R
    inp = {k: np.asarray(v) for k, v in R.setup_inputs().items()}
    out = kernel(**inp)
    exp = np.asarray(R.reference(**inp))
    err = np.abs(out - exp)
    print("absmax expected:", np.abs(exp).max())
    print("max abs err:", err.max(),
          "rel:", err.max() / np.abs(exp).max())
